# revision 35
# baseline (speedup 1.0000x reference)
"""Trainium2 Bass kernel: kNN(k=3) inverse-distance interpolation + 2-layer MLP.

Problem (hardcoded shapes):
  x        [4096, 256]  coarse features
  pos      [4096, 3]    coarse positions
  x_skip   [16384, 128] query (skip) features
  pos_skip [16384, 3]   query positions
  W1 [384,256] b1 [256] W2 [256,256] b2 [256]
  out = MLP(concat(knn_interpolate(x, pos -> pos_skip), x_skip))

Sharding: queries (M=16384) split across 8 cores (2048 each); coarse set and
weights replicated.  Per-core device program:
  - scores s = 2*q.p - |p|^2 via one bf16 K=128 matmul per [128,512] chunk
    (TensorE; fp32 accuracy recovered with a 3-way bf16 hi/lo split of the
    operands), so argmax_s == argmin_d2
  - top-8 per query row via DVE max / max_index (top-3 used) -- this 2-pass
    scan over [128, 4096] fp32 at ~1 elem/cycle/lane is the kernel's
    bottleneck engine (~135us/core)
  - weights w = 1/max(qq - s, 1e-6), normalized; chain on ScalarE, only the
    two reciprocals on DVE
  - gather 3 x-rows per query via indirect DMA ([128,1] offsets per call --
    hardware supports only one offset per partition)
  - weighted sum fused into the PE transpose by using rhs = diag(w_k)
    (PSUM accumulation over k) -> yT
  - MLP computed transposed (features on partitions, queries on free dim),
    biases/ReLU fused into the ScalarE PSUM->SBUF evictions
  - output written as outT [256, 2048]; host transposes/concats
"""

import sys

if "/opt/trn_rl_repo" not in sys.path:
    sys.path.insert(0, "/opt/trn_rl_repo")

import numpy as np

N, M, C_IN, C_SKIP, H = 4096, 16384, 256, 128, 256
K = 3
EPS = 1e-16
EPS2 = 1e-6    # reciprocal guard; never binds for this data (min d2_1 ~ 2e-5)
NCORES = 8
MS = M // NCORES          # queries per core
QT = MS // 128            # query tiles per core (16)
BW = 256                  # MLP block width (queries)
NBLK = MS // BW           # MLP blocks per core

_prog_cache = {}


def _build_program():
    import concourse.bass as bass
    import concourse.bacc as bacc
    import concourse.mybir as mybir
    from concourse.tile import TileContext

    f32 = mybir.dt.float32
    bf16 = mybir.dt.bfloat16
    u32 = mybir.dt.uint32
    Alu = mybir.AluOpType
    Act = mybir.ActivationFunctionType

    nc = bacc.Bacc(trn_type="TRN2")

    x_d = nc.dram_tensor("x", [N, C_IN], f32, kind="ExternalInput")
    # scores are computed in bf16 at K=128 (the only fast PE config: fp32 is
    # 2x cycles and K<128 another 2x).  fp32 accuracy is recovered by a 3-way
    # hi/lo/lo2 bf16 split of both position operands (6 exact cross-term
    # groups x 3 dims + 3 split |p|^2 rows = 21 live rows, zero padded).
    p4_d = nc.dram_tensor("p4", [128, N], bf16, kind="ExternalInput")
    q4_d = nc.dram_tensor("q4", [128, MS], bf16, kind="ExternalInput")
    qq_d = nc.dram_tensor("qq", [128, QT], f32, kind="ExternalInput")
    ident_d = nc.dram_tensor("ident", [128, 128], f32, kind="ExternalInput")
    eps2_d = nc.dram_tensor("eps2", [128, 1], f32, kind="ExternalInput")
    xskT_d = nc.dram_tensor("xskT", [C_SKIP, MS], f32, kind="ExternalInput")
    w1_d = nc.dram_tensor("W1", [C_IN + C_SKIP, H], f32, kind="ExternalInput")
    b1_d = nc.dram_tensor("b1", [128, H // 128], f32, kind="ExternalInput")
    w2_d = nc.dram_tensor("W2", [H, H], f32, kind="ExternalInput")
    b2_d = nc.dram_tensor("b2", [128, H // 128], f32, kind="ExternalInput")
    outT_d = nc.dram_tensor("outT", [H, MS], f32, kind="ExternalOutput")

    with TileContext(nc) as tc:
        with (
            tc.tile_pool(name="const", bufs=1) as cpool,
            tc.tile_pool(name="score", bufs=4) as spool,
            tc.tile_pool(name="small", bufs=6) as smpool,
            tc.tile_pool(name="gather", bufs=3) as gpool,
            tc.tile_pool(name="mlp", bufs=3) as mpool,
            tc.tile_pool(name="ps_score", bufs=2, space="PSUM") as ps_s,
            tc.tile_pool(name="ps_y", bufs=2, space="PSUM") as ps_y,
            tc.tile_pool(name="ps_mlp", bufs=2, space="PSUM") as ps_m,
        ):
            # ---- constants / resident tensors ----
            # only what query-tile 0's first score chunks need loads before
            # the barrier; everything else streams in behind the first scans
            p4_sb = cpool.tile([128, N], bf16, tag="p4")
            nc.sync.dma_start(out=p4_sb[:, 0:1024], in_=p4_d[:, 0:1024])
            q4_sb = cpool.tile([128, MS], bf16, tag="q4")
            nc.sync.dma_start(out=q4_sb[:, 0:128], in_=q4_d[:, 0:128])
            # The first PE instruction would otherwise need waits on two DMA
            # semaphore lanes; the LDWEIGHTS descriptor only supports one
            # sync-wait, so fence ONLY the two first-tile score chunks here
            # (fewer DMA completion round-trips before the barrier).
            tc.strict_bb_all_engine_barrier()

            nc.sync.dma_start(out=p4_sb[:, 1024:N], in_=p4_d[:, 1024:N])
            nc.sync.dma_start(out=q4_sb[:, 128:MS], in_=q4_d[:, 128:MS])
            qq_sb = cpool.tile([128, QT], f32, tag="qq")
            nc.sync.dma_start(out=qq_sb[:, :], in_=qq_d[:, :])
            ident = cpool.tile([128, 128], f32, tag="ident")
            nc.sync.dma_start(out=ident[:, :], in_=ident_d[:, :])
            eps2_sb = cpool.tile([128, 1], f32, tag="eps2")
            nc.sync.dma_start(out=eps2_sb[:, :], in_=eps2_d[:, :])
            xskT_sb = cpool.tile([C_SKIP, MS], f32, tag="xskT")
            nc.sync.dma_start(out=xskT_sb[:, :], in_=xskT_d[:, :])
            w1_sb = cpool.tile([128, 3, H], f32, tag="w1")
            for kc in range(3):
                nc.sync.dma_start(
                    out=w1_sb[:, kc, :], in_=w1_d[kc * 128 : (kc + 1) * 128, :]
                )
            w2_sb = cpool.tile([128, 2, H], f32, tag="w2")
            for kh in range(2):
                nc.sync.dma_start(
                    out=w2_sb[:, kh, :], in_=w2_d[kh * 128 : (kh + 1) * 128, :]
                )
            b1_sb = cpool.tile([128, 2], f32, tag="b1")
            nc.sync.dma_start(out=b1_sb[:, :], in_=b1_d[:, :])
            b2_sb = cpool.tile([128, 2], f32, tag="b2")
            nc.sync.dma_start(out=b2_sb[:, :], in_=b2_d[:, :])

            # per-block yT tiles (c-low chunk and c-high chunk of y transposed)
            yTa = [
                cpool.tile([128, BW], f32, tag=f"yTa{b}", name=f"yTa{b}")
                for b in range(NBLK)
            ]
            yTb = [
                cpool.tile([128, BW], f32, tag=f"yTb{b}", name=f"yTb{b}")
                for b in range(NBLK)
            ]

            # Desired DVE queue order (the scheduler otherwise hoists
            # MAX8(t+1) two tiles ahead and head-of-line blocks FIND):
            #   ... FIND(t), MAX8(t+1), recips(t), FIND(t+1) ...
            # pinned with no-sync scheduling edges via dve_prev.
            from concourse.tile import add_dep_helper

            dve_prev = {}

            def scan_tile(t):
                # scores for queries [t*128, (t+1)*128) against all N points
                s_sb = spool.tile([128, N], f32, tag="s")
                for q in range(4):
                    ps = ps_s.tile([128, 1024], f32, tag="ps")
                    for c in range(2):
                        col = (q * 2 + c) * 512
                        nc.tensor.matmul(
                            out=ps[:, c * 512 : (c + 1) * 512],
                            lhsT=q4_sb[:, t * 128 : (t + 1) * 128],
                            rhs=p4_sb[:, col : col + 512],
                            start=True,
                            stop=True,
                        )
                    nc.scalar.copy(
                        out=s_sb[:, q * 1024 : (q + 1) * 1024], in_=ps[:, :]
                    )

                v8 = smpool.tile([128, 8], f32, tag="v8")
                m_i = nc.vector.max(out=v8[:, :], in_=s_sb[:, :])
                if "find" in dve_prev:
                    # MAX8(t) after FIND(t-1)
                    add_dep_helper(m_i.ins, dve_prev["find"].ins, sync=False,
                                   reason="dve order: max8 after prev find")
                if "r1" in dve_prev:
                    # recip_w3(t-1) after MAX8(t) -- it hides the ScalarE
                    # round-trip of the weight chain under this MAX8
                    add_dep_helper(dve_prev["r1"].ins, m_i.ins, sync=False,
                                   reason="dve order: prev recip after max8")
                i8 = smpool.tile([128, 8], u32, tag="i8")
                f_i = nc.vector.max_index(
                    out=i8[:, :], in_max=v8[:, :], in_values=s_sb[:, :]
                )
                if "r2" in dve_prev:
                    # recip_sw(t-1) after FIND(t)
                    add_dep_helper(dve_prev["r2"].ins, f_i.ins, sync=False,
                                   reason="dve order: prev recip2 after find")
                dve_prev["find"] = f_i

                # inverse-distance weights from the top-3 scores.
                # d2 = max(qq - s, EPS2) computed as Relu(qq-EPS2 - s) + EPS2
                # (qq_sb holds qq-EPS2; EPS2 never binds on sane data, it just
                # guards 1/0).  Everything except the reciprocals runs on
                # ScalarE -- DVE is the bottleneck engine.
                w3 = smpool.tile([128, 3], f32, tag="w3")
                nc.scalar.activation(
                    out=w3[:, :], in_=v8[:, 0:3], func=Act.Relu,
                    bias=qq_sb[:, t : t + 1], scale=-1.0,
                )
                nc.scalar.activation(
                    out=w3[:, :], in_=w3[:, :], func=Act.Identity, bias=eps2_sb[:, 0:1],
                )
                r1 = nc.vector.reciprocal(w3[:, :], w3[:, :])
                sw = smpool.tile([128, 1], f32, tag="sw")
                nc.scalar.activation(
                    out=w3[:, :], in_=w3[:, :], func=Act.Identity,
                    accum_out=sw[:, :],
                )
                r2 = nc.vector.reciprocal(sw[:, :], sw[:, :])
                dve_prev["r1"], dve_prev["r2"] = r1, r2
                w3n = smpool.tile([128, 3], f32, tag="w3n")
                nc.scalar.activation(
                    out=w3n[:, :], in_=w3[:, :], func=Act.Identity,
                    scale=sw[:, 0:1],
                )

                # gather the 3 neighbor feature rows for each query.
                # NOTE: HW indirect DMA only supports one offset per
                # partition ([128,1] offset AP); CoreSim accepts [128,3]
                # but hardware produces garbage for it.
                gx = gpool.tile([128, 3, C_IN], f32, tag="gx")
                for k in range(K):
                    nc.gpsimd.indirect_dma_start(
                        out=gx[:, k, :],
                        out_offset=None,
                        in_=x_d[:, :],
                        in_offset=bass.IndirectOffsetOnAxis(
                            ap=i8[:, k : k + 1], axis=0
                        ),
                    )
                # weighted sum fused into PE transpose (PSUM accumulation):
                # yT = sum_k gx_k^T @ diag(w_k).  Building diag(w_k) from the
                # identity (per-partition scale) keeps ScalarE independent of
                # the gather, so its s-copies never stall behind gather DMAs.
                dk = smpool.tile([128, 3, 128], f32, tag="dk")
                for k in range(K):
                    nc.scalar.activation(
                        out=dk[:, k, :],
                        in_=ident[:, :],
                        func=Act.Copy,
                        scale=w3n[:, k : k + 1],
                    )
                py = ps_y.tile([128, 256], f32, tag="py")
                for h2 in range(2):
                    for k in range(K):
                        nc.tensor.matmul(
                            out=py[:, h2 * 128 : (h2 + 1) * 128],
                            lhsT=gx[:, k, h2 * 128 : (h2 + 1) * 128],
                            rhs=dk[:, k, :],
                            start=(k == 0),
                            stop=(k == K - 1),
                        )
                b, tq = t * 128 // BW, (t * 128 % BW) // 128
                nc.scalar.copy(
                    out=yTa[b][:, tq * 128 : (tq + 1) * 128], in_=py[:, 0:128]
                )
                nc.scalar.copy(
                    out=yTb[b][:, tq * 128 : (tq + 1) * 128], in_=py[:, 128:256]
                )

            def mlp_block(b):
                cols = slice(b * BW, (b + 1) * BW)
                h_sb = []
                for j in range(2):
                    ph = ps_m.tile([128, BW], f32, tag="pm")
                    for kc in range(3):
                        rhs = (
                            yTa[b][:, :]
                            if kc == 0
                            else yTb[b][:, :]
                            if kc == 1
                            else xskT_sb[:, cols]
                        )
                        nc.tensor.matmul(
                            out=ph[:, :],
                            lhsT=w1_sb[:, kc, j * 128 : (j + 1) * 128],
                            rhs=rhs,
                            start=(kc == 0),
                            stop=(kc == 2),
                        )
                    hj = mpool.tile([128, BW], f32, tag="h")
                    nc.scalar.activation(
                        out=hj[:, :], in_=ph[:, :], func=Act.Relu,
                        bias=b1_sb[:, j : j + 1],
                    )
                    h_sb.append(hj)
                for j2 in range(2):
                    po = ps_m.tile([128, BW], f32, tag="pm")
                    for kh in range(2):
                        nc.tensor.matmul(
                            out=po[:, :],
                            lhsT=w2_sb[:, kh, j2 * 128 : (j2 + 1) * 128],
                            rhs=h_sb[kh][:, :],
                            start=(kh == 0),
                            stop=(kh == 1),
                        )
                    ot = mpool.tile([128, BW], f32, tag="ot")
                    nc.scalar.activation(
                        out=ot[:, :], in_=po[:, :], func=Act.Identity,
                        bias=b2_sb[:, j2 : j2 + 1],
                    )
                    nc.sync.dma_start(
                        out=outT_d[j2 * 128 : (j2 + 1) * 128, cols], in_=ot[:, :]
                    )

            for t in range(QT):
                scan_tile(t)
                if (t + 1) * 128 % BW == 0:
                    mlp_block((t + 1) * 128 // BW - 1)

    nc.compile()
    return nc


def _host_prep(x, pos, x_skip, pos_skip, W1, b1, W2, b2):
    """Build the per-core input maps (all float32, layouts device expects)."""
    x = np.ascontiguousarray(np.asarray(x, dtype=np.float32))
    pos = np.asarray(pos, dtype=np.float32)
    x_skip = np.asarray(x_skip, dtype=np.float32)
    pos_skip = np.asarray(pos_skip, dtype=np.float32)
    W1 = np.ascontiguousarray(np.asarray(W1, dtype=np.float32))
    W2 = np.ascontiguousarray(np.asarray(W2, dtype=np.float32))
    b1 = np.asarray(b1, dtype=np.float32)
    b2 = np.asarray(b2, dtype=np.float32)

    import ml_dtypes

    bf = ml_dtypes.bfloat16

    def split3(v):
        h = v.astype(bf).astype(np.float32)
        r = v - h
        l1 = r.astype(bf).astype(np.float32)
        l2 = (r - l1).astype(bf).astype(np.float32)
        return h, l1, l2

    # moving table [128, N]: rows are p-side components for the 6 exact
    # cross-term groups (hh, h-l1, l1-h, h-l2, l2-h, l1-l1) plus split pp
    pp = np.sum(pos.astype(np.float64) * pos.astype(np.float64), axis=1).astype(
        np.float32
    )
    ph, pl1, pl2 = split3(pos)  # [N, 3] each
    pph, ppl1, ppl2 = split3(pp)  # [N]
    p4 = np.zeros((128, N), dtype=bf)
    p_rows = [ph, pl1, ph, pl2, ph, pl1]
    for g in range(6):
        p4[3 * g : 3 * g + 3, :] = p_rows[g].T.astype(bf)
    p4[18, :] = pph.astype(bf)
    p4[19, :] = ppl1.astype(bf)
    p4[20, :] = ppl2.astype(bf)
    p4 = np.ascontiguousarray(p4)

    b1_dev = np.ascontiguousarray(b1.reshape(2, 128).T)  # [128, 2]
    b2_dev = np.ascontiguousarray(b2.reshape(2, 128).T)

    ident_host = np.ascontiguousarray(np.eye(128, dtype=np.float32))
    eps2_host = np.full((128, 1), EPS2, dtype=np.float32)
    in_maps = []
    for c in range(NCORES):
        r0, r1 = c * MS, (c + 1) * MS
        ps = pos_skip[r0:r1]
        qh, ql1, ql2 = split3(2.0 * ps)  # [MS, 3] each
        q4 = np.zeros((128, MS), dtype=bf)
        q_rows = [qh, qh, ql1, qh, ql2, ql1]
        for g in range(6):
            q4[3 * g : 3 * g + 3, :] = q_rows[g].T.astype(bf)
        q4[18:21, :] = np.float32(-1.0)
        q4 = np.ascontiguousarray(q4)
        qq = np.ascontiguousarray(
            (np.sum(ps * ps, axis=1) - EPS2).reshape(QT, 128).T
        )  # [128, QT], minus the reciprocal-guard epsilon
        xskT = np.ascontiguousarray(x_skip[r0:r1].T)  # [C_SKIP, MS]
        in_maps.append(
            {
                "x": x,
                "p4": p4,
                "q4": q4,
                "qq": qq,
                "xskT": xskT,
                "W1": W1,
                "b1": b1_dev,
                "W2": W2,
                "b2": b2_dev,
                "ident": ident_host,
                "eps2": eps2_host,
            }
        )
    return in_maps


def run(inputs, trace=False):
    """Run the bass kernel on 8 cores. Returns (out [M, C_IN... H], results)."""
    from concourse.bass_utils import run_bass_kernel_spmd

    if "nc" not in _prog_cache:
        _prog_cache["nc"] = _build_program()
    nc = _prog_cache["nc"]

    in_maps = _host_prep(
        inputs["x"], inputs["pos"], inputs["x_skip"], inputs["pos_skip"],
        inputs["W1"], inputs["b1"], inputs["W2"], inputs["b2"],
    )
    try:
        res = run_bass_kernel_spmd(
            nc, in_maps, core_ids=list(range(NCORES)), trace=trace
        )
    except Exception:
        # transient axon device wedge (NRT_EXEC_UNIT_UNRECOVERABLE) --
        # reset the terminal-side NRT state and retry once
        try:
            import ctypes

            lib = ctypes.CDLL("/opt/axon/libaxon_pjrt.so")
            lib.axon_reset.restype = ctypes.c_int64
            lib.axon_reset()
        except Exception:
            pass
        res = run_bass_kernel_spmd(
            nc, in_maps, core_ids=list(range(NCORES)), trace=False
        )
    out = np.concatenate(
        [np.ascontiguousarray(r["outT"].T) for r in res.results], axis=0
    )
    return out, res


def kernel(x, pos, batch, x_skip, pos_skip, batch_skip, W1, b1, W2, b2):
    out, _ = run(
        {
            "x": x, "pos": pos, "x_skip": x_skip, "pos_skip": pos_skip,
            "W1": W1, "b1": b1, "W2": W2, "b2": b2,
        }
    )
    return (
        out,
        np.asarray(pos_skip, dtype=np.float32),
        np.asarray(batch_skip),
    )


# revision 36
# speedup vs baseline: 1.0177x; 1.0177x over previous
"""Trainium2 Bass kernel: kNN(k=3) inverse-distance interpolation + 2-layer MLP.

Problem (hardcoded shapes):
  x        [4096, 256]  coarse features
  pos      [4096, 3]    coarse positions
  x_skip   [16384, 128] query (skip) features
  pos_skip [16384, 3]   query positions
  W1 [384,256] b1 [256] W2 [256,256] b2 [256]
  out = MLP(concat(knn_interpolate(x, pos -> pos_skip), x_skip))

Sharding: queries (M=16384) split across 8 cores (2048 each); coarse set and
weights replicated.  Per-core device program:
  - scores s = 2*q.p - |p|^2 via one bf16 K=128 matmul per [128,512] chunk
    (TensorE; fp32 accuracy recovered with a 3-way bf16 hi/lo split of the
    operands), so argmax_s == argmin_d2
  - top-8 per query row via DVE max / max_index (top-3 used) -- this 2-pass
    scan over [128, 4096] fp32 at ~1 elem/cycle/lane is the kernel's
    bottleneck engine (~135us/core)
  - weights w = 1/max(qq - s, 1e-6), normalized; chain on ScalarE, only the
    two reciprocals on DVE
  - gather 3 x-rows per query via indirect DMA ([128,1] offsets per call --
    hardware supports only one offset per partition)
  - weighted sum fused into the PE transpose by using rhs = diag(w_k)
    (PSUM accumulation over k) -> yT
  - MLP computed transposed (features on partitions, queries on free dim),
    biases/ReLU fused into the ScalarE PSUM->SBUF evictions
  - output written as outT [256, 2048]; host transposes/concats
"""

import sys

if "/opt/trn_rl_repo" not in sys.path:
    sys.path.insert(0, "/opt/trn_rl_repo")

import numpy as np

N, M, C_IN, C_SKIP, H = 4096, 16384, 256, 128, 256
K = 3
EPS = 1e-16
EPS2 = 1e-6    # reciprocal guard; never binds for this data (min d2_1 ~ 2e-5)
NCORES = 8
MS = M // NCORES          # queries per core
QT = MS // 128            # query tiles per core (16)
BW = 256                  # MLP block width (queries)
NBLK = MS // BW           # MLP blocks per core

_prog_cache = {}


def _build_program():
    import concourse.bass as bass
    import concourse.bacc as bacc
    import concourse.mybir as mybir
    from concourse.tile import TileContext

    f32 = mybir.dt.float32
    bf16 = mybir.dt.bfloat16
    u32 = mybir.dt.uint32
    Alu = mybir.AluOpType
    Act = mybir.ActivationFunctionType

    nc = bacc.Bacc(trn_type="TRN2")

    x_d = nc.dram_tensor("x", [N, C_IN], f32, kind="ExternalInput")
    # scores are computed in bf16 at K=128 (the only fast PE config: fp32 is
    # 2x cycles and K<128 another 2x).  fp32 accuracy is recovered by a 3-way
    # hi/lo/lo2 bf16 split of both position operands (6 exact cross-term
    # groups x 3 dims + 3 split |p|^2 rows = 21 live rows, zero padded).
    p4_d = nc.dram_tensor("p4", [128, N], bf16, kind="ExternalInput")
    q4_d = nc.dram_tensor("q4", [128, MS], bf16, kind="ExternalInput")
    qq_d = nc.dram_tensor("qq", [128, QT], f32, kind="ExternalInput")
    ident_d = nc.dram_tensor("ident", [128, 128], f32, kind="ExternalInput")
    eps2_d = nc.dram_tensor("eps2", [128, 1], f32, kind="ExternalInput")
    xskT_d = nc.dram_tensor("xskT", [C_SKIP, MS], f32, kind="ExternalInput")
    w1_d = nc.dram_tensor("W1", [C_IN + C_SKIP, H], f32, kind="ExternalInput")
    b1_d = nc.dram_tensor("b1", [128, H // 128], f32, kind="ExternalInput")
    w2_d = nc.dram_tensor("W2", [H, H], f32, kind="ExternalInput")
    b2_d = nc.dram_tensor("b2", [128, H // 128], f32, kind="ExternalInput")
    outT_d = nc.dram_tensor("outT", [H, MS], f32, kind="ExternalOutput")

    with TileContext(nc) as tc:
        with (
            tc.tile_pool(name="const", bufs=1) as cpool,
            tc.tile_pool(name="score", bufs=4) as spool,
            tc.tile_pool(name="small", bufs=6) as smpool,
            tc.tile_pool(name="gather", bufs=3) as gpool,
            tc.tile_pool(name="mlp", bufs=3) as mpool,
            tc.tile_pool(name="ps_score", bufs=2, space="PSUM") as ps_s,
            tc.tile_pool(name="ps_y", bufs=2, space="PSUM") as ps_y,
            tc.tile_pool(name="ps_mlp", bufs=2, space="PSUM") as ps_m,
        ):
            # ---- constants / resident tensors ----
            # only what query-tile 0's first score chunks need loads before
            # the barrier; everything else streams in behind the first scans
            p4_sb = cpool.tile([128, N], bf16, tag="p4")
            nc.sync.dma_start(out=p4_sb[:, 0:1024], in_=p4_d[:, 0:1024])
            q4_sb = cpool.tile([128, MS], bf16, tag="q4")
            nc.sync.dma_start(out=q4_sb[:, 0:128], in_=q4_d[:, 0:128])
            nc.sync.dma_start(out=p4_sb[:, 1024:N], in_=p4_d[:, 1024:N])
            nc.sync.dma_start(out=q4_sb[:, 128:MS], in_=q4_d[:, 128:MS])
            qq_sb = cpool.tile([128, QT], f32, tag="qq")
            nc.sync.dma_start(out=qq_sb[:, :], in_=qq_d[:, :])
            ident = cpool.tile([128, 128], f32, tag="ident")
            nc.sync.dma_start(out=ident[:, :], in_=ident_d[:, :])
            eps2_sb = cpool.tile([128, 1], f32, tag="eps2")
            nc.sync.dma_start(out=eps2_sb[:, :], in_=eps2_d[:, :])
            xskT_sb = cpool.tile([C_SKIP, MS], f32, tag="xskT")
            nc.sync.dma_start(out=xskT_sb[:, :], in_=xskT_d[:, :])
            w1_sb = cpool.tile([128, 3, H], f32, tag="w1")
            for kc in range(3):
                nc.sync.dma_start(
                    out=w1_sb[:, kc, :], in_=w1_d[kc * 128 : (kc + 1) * 128, :]
                )
            w2_sb = cpool.tile([128, 2, H], f32, tag="w2")
            for kh in range(2):
                nc.sync.dma_start(
                    out=w2_sb[:, kh, :], in_=w2_d[kh * 128 : (kh + 1) * 128, :]
                )
            b1_sb = cpool.tile([128, 2], f32, tag="b1")
            nc.sync.dma_start(out=b1_sb[:, :], in_=b1_d[:, :])
            b2_sb = cpool.tile([128, 2], f32, tag="b2")
            nc.sync.dma_start(out=b2_sb[:, :], in_=b2_d[:, :])

            # per-block yT tiles (c-low chunk and c-high chunk of y transposed)
            yTa = [
                cpool.tile([128, BW], f32, tag=f"yTa{b}", name=f"yTa{b}")
                for b in range(NBLK)
            ]
            yTb = [
                cpool.tile([128, BW], f32, tag=f"yTb{b}", name=f"yTb{b}")
                for b in range(NBLK)
            ]

            # Desired DVE queue order (the scheduler otherwise hoists
            # MAX8(t+1) two tiles ahead and head-of-line blocks FIND):
            #   ... FIND(t), MAX8(t+1), recips(t), FIND(t+1) ...
            # pinned with no-sync scheduling edges via dve_prev.
            from concourse.tile import add_dep_helper

            dve_prev = {}

            def scan_tile(t):
                # scores for queries [t*128, (t+1)*128) against all N points
                s_sb = spool.tile([128, N], f32, tag="s")
                for q in range(4):
                    ps = ps_s.tile([128, 1024], f32, tag="ps")
                    for c in range(2):
                        col = (q * 2 + c) * 512
                        nc.tensor.matmul(
                            out=ps[:, c * 512 : (c + 1) * 512],
                            lhsT=q4_sb[:, t * 128 : (t + 1) * 128],
                            rhs=p4_sb[:, col : col + 512],
                            start=True,
                            stop=True,
                        )
                    nc.scalar.copy(
                        out=s_sb[:, q * 1024 : (q + 1) * 1024], in_=ps[:, :]
                    )

                v8 = smpool.tile([128, 8], f32, tag="v8")
                m_i = nc.vector.max(out=v8[:, :], in_=s_sb[:, :])
                if "find" in dve_prev:
                    # MAX8(t) after FIND(t-1)
                    add_dep_helper(m_i.ins, dve_prev["find"].ins, sync=False,
                                   reason="dve order: max8 after prev find")
                if "r1" in dve_prev:
                    # recip_w3(t-1) after MAX8(t) -- it hides the ScalarE
                    # round-trip of the weight chain under this MAX8
                    add_dep_helper(dve_prev["r1"].ins, m_i.ins, sync=False,
                                   reason="dve order: prev recip after max8")
                i8 = smpool.tile([128, 8], u32, tag="i8")
                f_i = nc.vector.max_index(
                    out=i8[:, :], in_max=v8[:, :], in_values=s_sb[:, :]
                )
                if "r2" in dve_prev:
                    # recip_sw(t-1) after FIND(t)
                    add_dep_helper(dve_prev["r2"].ins, f_i.ins, sync=False,
                                   reason="dve order: prev recip2 after find")
                dve_prev["find"] = f_i

                # inverse-distance weights from the top-3 scores.
                # d2 = max(qq - s, EPS2) computed as Relu(qq-EPS2 - s) + EPS2
                # (qq_sb holds qq-EPS2; EPS2 never binds on sane data, it just
                # guards 1/0).  Everything except the reciprocals runs on
                # ScalarE -- DVE is the bottleneck engine.
                w3 = smpool.tile([128, 3], f32, tag="w3")
                nc.scalar.activation(
                    out=w3[:, :], in_=v8[:, 0:3], func=Act.Relu,
                    bias=qq_sb[:, t : t + 1], scale=-1.0,
                )
                nc.scalar.activation(
                    out=w3[:, :], in_=w3[:, :], func=Act.Identity, bias=eps2_sb[:, 0:1],
                )
                r1 = nc.vector.reciprocal(w3[:, :], w3[:, :])
                sw = smpool.tile([128, 1], f32, tag="sw")
                nc.scalar.activation(
                    out=w3[:, :], in_=w3[:, :], func=Act.Identity,
                    accum_out=sw[:, :],
                )
                r2 = nc.vector.reciprocal(sw[:, :], sw[:, :])
                dve_prev["r1"], dve_prev["r2"] = r1, r2
                w3n = smpool.tile([128, 3], f32, tag="w3n")
                nc.scalar.activation(
                    out=w3n[:, :], in_=w3[:, :], func=Act.Identity,
                    scale=sw[:, 0:1],
                )

                # gather the 3 neighbor feature rows for each query.
                # NOTE: HW indirect DMA only supports one offset per
                # partition ([128,1] offset AP); CoreSim accepts [128,3]
                # but hardware produces garbage for it.
                gx = gpool.tile([128, 3, C_IN], f32, tag="gx")
                for k in range(K):
                    nc.gpsimd.indirect_dma_start(
                        out=gx[:, k, :],
                        out_offset=None,
                        in_=x_d[:, :],
                        in_offset=bass.IndirectOffsetOnAxis(
                            ap=i8[:, k : k + 1], axis=0
                        ),
                    )
                # weighted sum fused into PE transpose (PSUM accumulation):
                # yT = sum_k gx_k^T @ diag(w_k).  Building diag(w_k) from the
                # identity (per-partition scale) keeps ScalarE independent of
                # the gather, so its s-copies never stall behind gather DMAs.
                dk = smpool.tile([128, 3, 128], f32, tag="dk")
                for k in range(K):
                    nc.scalar.activation(
                        out=dk[:, k, :],
                        in_=ident[:, :],
                        func=Act.Copy,
                        scale=w3n[:, k : k + 1],
                    )
                py = ps_y.tile([128, 256], f32, tag="py")
                for h2 in range(2):
                    for k in range(K):
                        nc.tensor.matmul(
                            out=py[:, h2 * 128 : (h2 + 1) * 128],
                            lhsT=gx[:, k, h2 * 128 : (h2 + 1) * 128],
                            rhs=dk[:, k, :],
                            start=(k == 0),
                            stop=(k == K - 1),
                        )
                b, tq = t * 128 // BW, (t * 128 % BW) // 128
                nc.scalar.copy(
                    out=yTa[b][:, tq * 128 : (tq + 1) * 128], in_=py[:, 0:128]
                )
                nc.scalar.copy(
                    out=yTb[b][:, tq * 128 : (tq + 1) * 128], in_=py[:, 128:256]
                )

            def mlp_block(b):
                cols = slice(b * BW, (b + 1) * BW)
                h_sb = []
                for j in range(2):
                    ph = ps_m.tile([128, BW], f32, tag="pm")
                    for kc in range(3):
                        rhs = (
                            yTa[b][:, :]
                            if kc == 0
                            else yTb[b][:, :]
                            if kc == 1
                            else xskT_sb[:, cols]
                        )
                        nc.tensor.matmul(
                            out=ph[:, :],
                            lhsT=w1_sb[:, kc, j * 128 : (j + 1) * 128],
                            rhs=rhs,
                            start=(kc == 0),
                            stop=(kc == 2),
                        )
                    hj = mpool.tile([128, BW], f32, tag="h")
                    nc.scalar.activation(
                        out=hj[:, :], in_=ph[:, :], func=Act.Relu,
                        bias=b1_sb[:, j : j + 1],
                    )
                    h_sb.append(hj)
                for j2 in range(2):
                    po = ps_m.tile([128, BW], f32, tag="pm")
                    for kh in range(2):
                        nc.tensor.matmul(
                            out=po[:, :],
                            lhsT=w2_sb[:, kh, j2 * 128 : (j2 + 1) * 128],
                            rhs=h_sb[kh][:, :],
                            start=(kh == 0),
                            stop=(kh == 1),
                        )
                    ot = mpool.tile([128, BW], f32, tag="ot")
                    nc.scalar.activation(
                        out=ot[:, :], in_=po[:, :], func=Act.Identity,
                        bias=b2_sb[:, j2 : j2 + 1],
                    )
                    nc.sync.dma_start(
                        out=outT_d[j2 * 128 : (j2 + 1) * 128, cols], in_=ot[:, :]
                    )

            for t in range(QT):
                scan_tile(t)
                if (t + 1) * 128 % BW == 0:
                    mlp_block((t + 1) * 128 // BW - 1)

    nc.compile()
    return nc


def _host_prep(x, pos, x_skip, pos_skip, W1, b1, W2, b2):
    """Build the per-core input maps (all float32, layouts device expects)."""
    x = np.ascontiguousarray(np.asarray(x, dtype=np.float32))
    pos = np.asarray(pos, dtype=np.float32)
    x_skip = np.asarray(x_skip, dtype=np.float32)
    pos_skip = np.asarray(pos_skip, dtype=np.float32)
    W1 = np.ascontiguousarray(np.asarray(W1, dtype=np.float32))
    W2 = np.ascontiguousarray(np.asarray(W2, dtype=np.float32))
    b1 = np.asarray(b1, dtype=np.float32)
    b2 = np.asarray(b2, dtype=np.float32)

    import ml_dtypes

    bf = ml_dtypes.bfloat16

    def split3(v):
        h = v.astype(bf).astype(np.float32)
        r = v - h
        l1 = r.astype(bf).astype(np.float32)
        l2 = (r - l1).astype(bf).astype(np.float32)
        return h, l1, l2

    # moving table [128, N]: rows are p-side components for the 6 exact
    # cross-term groups (hh, h-l1, l1-h, h-l2, l2-h, l1-l1) plus split pp
    pp = np.sum(pos.astype(np.float64) * pos.astype(np.float64), axis=1).astype(
        np.float32
    )
    ph, pl1, pl2 = split3(pos)  # [N, 3] each
    pph, ppl1, ppl2 = split3(pp)  # [N]
    p4 = np.zeros((128, N), dtype=bf)
    p_rows = [ph, pl1, ph, pl2, ph, pl1]
    for g in range(6):
        p4[3 * g : 3 * g + 3, :] = p_rows[g].T.astype(bf)
    p4[18, :] = pph.astype(bf)
    p4[19, :] = ppl1.astype(bf)
    p4[20, :] = ppl2.astype(bf)
    p4 = np.ascontiguousarray(p4)

    b1_dev = np.ascontiguousarray(b1.reshape(2, 128).T)  # [128, 2]
    b2_dev = np.ascontiguousarray(b2.reshape(2, 128).T)

    ident_host = np.ascontiguousarray(np.eye(128, dtype=np.float32))
    eps2_host = np.full((128, 1), EPS2, dtype=np.float32)
    in_maps = []
    for c in range(NCORES):
        r0, r1 = c * MS, (c + 1) * MS
        ps = pos_skip[r0:r1]
        qh, ql1, ql2 = split3(2.0 * ps)  # [MS, 3] each
        q4 = np.zeros((128, MS), dtype=bf)
        q_rows = [qh, qh, ql1, qh, ql2, ql1]
        for g in range(6):
            q4[3 * g : 3 * g + 3, :] = q_rows[g].T.astype(bf)
        q4[18:21, :] = np.float32(-1.0)
        q4 = np.ascontiguousarray(q4)
        qq = np.ascontiguousarray(
            (np.sum(ps * ps, axis=1) - EPS2).reshape(QT, 128).T
        )  # [128, QT], minus the reciprocal-guard epsilon
        xskT = np.ascontiguousarray(x_skip[r0:r1].T)  # [C_SKIP, MS]
        in_maps.append(
            {
                "x": x,
                "p4": p4,
                "q4": q4,
                "qq": qq,
                "xskT": xskT,
                "W1": W1,
                "b1": b1_dev,
                "W2": W2,
                "b2": b2_dev,
                "ident": ident_host,
                "eps2": eps2_host,
            }
        )
    return in_maps


def run(inputs, trace=False):
    """Run the bass kernel on 8 cores. Returns (out [M, C_IN... H], results)."""
    from concourse.bass_utils import run_bass_kernel_spmd

    if "nc" not in _prog_cache:
        _prog_cache["nc"] = _build_program()
    nc = _prog_cache["nc"]

    in_maps = _host_prep(
        inputs["x"], inputs["pos"], inputs["x_skip"], inputs["pos_skip"],
        inputs["W1"], inputs["b1"], inputs["W2"], inputs["b2"],
    )
    try:
        res = run_bass_kernel_spmd(
            nc, in_maps, core_ids=list(range(NCORES)), trace=trace
        )
    except Exception:
        # transient axon device wedge (NRT_EXEC_UNIT_UNRECOVERABLE) --
        # reset the terminal-side NRT state and retry once
        try:
            import ctypes

            lib = ctypes.CDLL("/opt/axon/libaxon_pjrt.so")
            lib.axon_reset.restype = ctypes.c_int64
            lib.axon_reset()
        except Exception:
            pass
        res = run_bass_kernel_spmd(
            nc, in_maps, core_ids=list(range(NCORES)), trace=False
        )
    out = np.concatenate(
        [np.ascontiguousarray(r["outT"].T) for r in res.results], axis=0
    )
    return out, res


def kernel(x, pos, batch, x_skip, pos_skip, batch_skip, W1, b1, W2, b2):
    out, _ = run(
        {
            "x": x, "pos": pos, "x_skip": x_skip, "pos_skip": pos_skip,
            "W1": W1, "b1": b1, "W2": W2, "b2": b2,
        }
    )
    return (
        out,
        np.asarray(pos_skip, dtype=np.float32),
        np.asarray(batch_skip),
    )


# revision 37
# speedup vs baseline: 1.0179x; 1.0002x over previous
"""Trainium2 Bass kernel: kNN(k=3) inverse-distance interpolation + 2-layer MLP.

Problem (hardcoded shapes):
  x        [4096, 256]  coarse features
  pos      [4096, 3]    coarse positions
  x_skip   [16384, 128] query (skip) features
  pos_skip [16384, 3]   query positions
  W1 [384,256] b1 [256] W2 [256,256] b2 [256]
  out = MLP(concat(knn_interpolate(x, pos -> pos_skip), x_skip))

Sharding: queries (M=16384) split across 8 cores (2048 each); coarse set and
weights replicated.  Per-core device program:
  - scores s = 2*q.p - |p|^2 via one bf16 K=128 matmul per [128,512] chunk
    (TensorE; fp32 accuracy recovered with a 3-way bf16 hi/lo split of the
    operands), so argmax_s == argmin_d2
  - top-8 per query row via DVE max / max_index (top-3 used) -- this 2-pass
    scan over [128, 4096] fp32 at ~1 elem/cycle/lane is the kernel's
    bottleneck engine (~135us/core)
  - weights w = 1/max(qq - s, 1e-6), normalized; chain on ScalarE, only the
    two reciprocals on DVE
  - gather 3 x-rows per query via indirect DMA ([128,1] offsets per call --
    hardware supports only one offset per partition)
  - weighted sum fused into the PE transpose by using rhs = diag(w_k)
    (PSUM accumulation over k) -> yT
  - MLP computed transposed (features on partitions, queries on free dim),
    biases/ReLU fused into the ScalarE PSUM->SBUF evictions
  - output written as outT [256, 2048]; host transposes/concats
"""

import sys

if "/opt/trn_rl_repo" not in sys.path:
    sys.path.insert(0, "/opt/trn_rl_repo")

import numpy as np

N, M, C_IN, C_SKIP, H = 4096, 16384, 256, 128, 256
K = 3
EPS = 1e-16
EPS2 = 1e-6    # reciprocal guard; never binds for this data (min d2_1 ~ 2e-5)
NCORES = 8
MS = M // NCORES          # queries per core
QT = MS // 128            # query tiles per core (16)
BW = 256                  # MLP block width (queries)
NBLK = MS // BW           # MLP blocks per core

_prog_cache = {}


def _build_program():
    import concourse.bass as bass
    import concourse.bacc as bacc
    import concourse.mybir as mybir
    from concourse.tile import TileContext

    f32 = mybir.dt.float32
    bf16 = mybir.dt.bfloat16
    u32 = mybir.dt.uint32
    Alu = mybir.AluOpType
    Act = mybir.ActivationFunctionType

    nc = bacc.Bacc(trn_type="TRN2", num_swdge_queues=2)

    x_d = nc.dram_tensor("x", [N, C_IN], f32, kind="ExternalInput")
    # scores are computed in bf16 at K=128 (the only fast PE config: fp32 is
    # 2x cycles and K<128 another 2x).  fp32 accuracy is recovered by a 3-way
    # hi/lo/lo2 bf16 split of both position operands (6 exact cross-term
    # groups x 3 dims + 3 split |p|^2 rows = 21 live rows, zero padded).
    p4_d = nc.dram_tensor("p4", [128, N], bf16, kind="ExternalInput")
    q4_d = nc.dram_tensor("q4", [128, MS], bf16, kind="ExternalInput")
    qq_d = nc.dram_tensor("qq", [128, QT], f32, kind="ExternalInput")
    ident_d = nc.dram_tensor("ident", [128, 128], f32, kind="ExternalInput")
    eps2_d = nc.dram_tensor("eps2", [128, 1], f32, kind="ExternalInput")
    xskT_d = nc.dram_tensor("xskT", [C_SKIP, MS], f32, kind="ExternalInput")
    w1_d = nc.dram_tensor("W1", [C_IN + C_SKIP, H], f32, kind="ExternalInput")
    b1_d = nc.dram_tensor("b1", [128, H // 128], f32, kind="ExternalInput")
    w2_d = nc.dram_tensor("W2", [H, H], f32, kind="ExternalInput")
    b2_d = nc.dram_tensor("b2", [128, H // 128], f32, kind="ExternalInput")
    outT_d = nc.dram_tensor("outT", [H, MS], f32, kind="ExternalOutput")

    with TileContext(nc) as tc:
        with (
            tc.tile_pool(name="const", bufs=1) as cpool,
            tc.tile_pool(name="score", bufs=4) as spool,
            tc.tile_pool(name="small", bufs=6) as smpool,
            tc.tile_pool(name="gather", bufs=3) as gpool,
            tc.tile_pool(name="mlp", bufs=3) as mpool,
            tc.tile_pool(name="ps_score", bufs=2, space="PSUM") as ps_s,
            tc.tile_pool(name="ps_y", bufs=2, space="PSUM") as ps_y,
            tc.tile_pool(name="ps_mlp", bufs=2, space="PSUM") as ps_m,
        ):
            # ---- constants / resident tensors ----
            # only what query-tile 0's first score chunks need loads before
            # the barrier; everything else streams in behind the first scans
            p4_sb = cpool.tile([128, N], bf16, tag="p4")
            nc.sync.dma_start(out=p4_sb[:, 0:1024], in_=p4_d[:, 0:1024])
            q4_sb = cpool.tile([128, MS], bf16, tag="q4")
            nc.sync.dma_start(out=q4_sb[:, 0:128], in_=q4_d[:, 0:128])
            nc.sync.dma_start(out=p4_sb[:, 1024:N], in_=p4_d[:, 1024:N])
            nc.sync.dma_start(out=q4_sb[:, 128:MS], in_=q4_d[:, 128:MS])
            qq_sb = cpool.tile([128, QT], f32, tag="qq")
            nc.sync.dma_start(out=qq_sb[:, :], in_=qq_d[:, :])
            ident = cpool.tile([128, 128], f32, tag="ident")
            nc.sync.dma_start(out=ident[:, :], in_=ident_d[:, :])
            eps2_sb = cpool.tile([128, 1], f32, tag="eps2")
            nc.sync.dma_start(out=eps2_sb[:, :], in_=eps2_d[:, :])
            xskT_sb = cpool.tile([C_SKIP, MS], f32, tag="xskT")
            nc.sync.dma_start(out=xskT_sb[:, :], in_=xskT_d[:, :])
            w1_sb = cpool.tile([128, 3, H], f32, tag="w1")
            for kc in range(3):
                nc.sync.dma_start(
                    out=w1_sb[:, kc, :], in_=w1_d[kc * 128 : (kc + 1) * 128, :]
                )
            w2_sb = cpool.tile([128, 2, H], f32, tag="w2")
            for kh in range(2):
                nc.sync.dma_start(
                    out=w2_sb[:, kh, :], in_=w2_d[kh * 128 : (kh + 1) * 128, :]
                )
            b1_sb = cpool.tile([128, 2], f32, tag="b1")
            nc.sync.dma_start(out=b1_sb[:, :], in_=b1_d[:, :])
            b2_sb = cpool.tile([128, 2], f32, tag="b2")
            nc.sync.dma_start(out=b2_sb[:, :], in_=b2_d[:, :])

            # per-block yT tiles (c-low chunk and c-high chunk of y transposed)
            yTa = [
                cpool.tile([128, BW], f32, tag=f"yTa{b}", name=f"yTa{b}")
                for b in range(NBLK)
            ]
            yTb = [
                cpool.tile([128, BW], f32, tag=f"yTb{b}", name=f"yTb{b}")
                for b in range(NBLK)
            ]

            # Desired DVE queue order (the scheduler otherwise hoists
            # MAX8(t+1) two tiles ahead and head-of-line blocks FIND):
            #   ... FIND(t), MAX8(t+1), recips(t), FIND(t+1) ...
            # pinned with no-sync scheduling edges via dve_prev.
            from concourse.tile import add_dep_helper

            dve_prev = {}

            def scan_tile(t):
                # scores for queries [t*128, (t+1)*128) against all N points
                s_sb = spool.tile([128, N], f32, tag="s")
                for q in range(4):
                    ps = ps_s.tile([128, 1024], f32, tag="ps")
                    for c in range(2):
                        col = (q * 2 + c) * 512
                        nc.tensor.matmul(
                            out=ps[:, c * 512 : (c + 1) * 512],
                            lhsT=q4_sb[:, t * 128 : (t + 1) * 128],
                            rhs=p4_sb[:, col : col + 512],
                            start=True,
                            stop=True,
                        )
                    nc.scalar.copy(
                        out=s_sb[:, q * 1024 : (q + 1) * 1024], in_=ps[:, :]
                    )

                v8 = smpool.tile([128, 8], f32, tag="v8")
                m_i = nc.vector.max(out=v8[:, :], in_=s_sb[:, :])
                if "find" in dve_prev:
                    # MAX8(t) after FIND(t-1)
                    add_dep_helper(m_i.ins, dve_prev["find"].ins, sync=False,
                                   reason="dve order: max8 after prev find")
                if "r1" in dve_prev:
                    # recip_w3(t-1) after MAX8(t) -- it hides the ScalarE
                    # round-trip of the weight chain under this MAX8
                    add_dep_helper(dve_prev["r1"].ins, m_i.ins, sync=False,
                                   reason="dve order: prev recip after max8")
                i8 = smpool.tile([128, 8], u32, tag="i8")
                f_i = nc.vector.max_index(
                    out=i8[:, :], in_max=v8[:, :], in_values=s_sb[:, :]
                )
                if "r2" in dve_prev:
                    # recip_sw(t-1) after FIND(t)
                    add_dep_helper(dve_prev["r2"].ins, f_i.ins, sync=False,
                                   reason="dve order: prev recip2 after find")
                dve_prev["find"] = f_i

                # inverse-distance weights from the top-3 scores.
                # d2 = max(qq - s, EPS2) computed as Relu(qq-EPS2 - s) + EPS2
                # (qq_sb holds qq-EPS2; EPS2 never binds on sane data, it just
                # guards 1/0).  Everything except the reciprocals runs on
                # ScalarE -- DVE is the bottleneck engine.
                w3 = smpool.tile([128, 3], f32, tag="w3")
                nc.scalar.activation(
                    out=w3[:, :], in_=v8[:, 0:3], func=Act.Relu,
                    bias=qq_sb[:, t : t + 1], scale=-1.0,
                )
                nc.scalar.activation(
                    out=w3[:, :], in_=w3[:, :], func=Act.Identity, bias=eps2_sb[:, 0:1],
                )
                r1 = nc.vector.reciprocal(w3[:, :], w3[:, :])
                sw = smpool.tile([128, 1], f32, tag="sw")
                nc.scalar.activation(
                    out=w3[:, :], in_=w3[:, :], func=Act.Identity,
                    accum_out=sw[:, :],
                )
                r2 = nc.vector.reciprocal(sw[:, :], sw[:, :])
                dve_prev["r1"], dve_prev["r2"] = r1, r2
                w3n = smpool.tile([128, 3], f32, tag="w3n")
                nc.scalar.activation(
                    out=w3n[:, :], in_=w3[:, :], func=Act.Identity,
                    scale=sw[:, 0:1],
                )

                # gather the 3 neighbor feature rows for each query.
                # NOTE: HW indirect DMA only supports one offset per
                # partition ([128,1] offset AP); CoreSim accepts [128,3]
                # but hardware produces garbage for it.
                gx = gpool.tile([128, 3, C_IN], f32, tag="gx")
                for k in range(K):
                    nc.gpsimd.indirect_dma_start(
                        out=gx[:, k, :],
                        out_offset=None,
                        in_=x_d[:, :],
                        in_offset=bass.IndirectOffsetOnAxis(
                            ap=i8[:, k : k + 1], axis=0
                        ),
                    )
                # weighted sum fused into PE transpose (PSUM accumulation):
                # yT = sum_k gx_k^T @ diag(w_k).  Building diag(w_k) from the
                # identity (per-partition scale) keeps ScalarE independent of
                # the gather, so its s-copies never stall behind gather DMAs.
                dk = smpool.tile([128, 3, 128], f32, tag="dk")
                for k in range(K):
                    nc.scalar.activation(
                        out=dk[:, k, :],
                        in_=ident[:, :],
                        func=Act.Copy,
                        scale=w3n[:, k : k + 1],
                    )
                py = ps_y.tile([128, 256], f32, tag="py")
                for h2 in range(2):
                    for k in range(K):
                        nc.tensor.matmul(
                            out=py[:, h2 * 128 : (h2 + 1) * 128],
                            lhsT=gx[:, k, h2 * 128 : (h2 + 1) * 128],
                            rhs=dk[:, k, :],
                            start=(k == 0),
                            stop=(k == K - 1),
                        )
                b, tq = t * 128 // BW, (t * 128 % BW) // 128
                nc.scalar.copy(
                    out=yTa[b][:, tq * 128 : (tq + 1) * 128], in_=py[:, 0:128]
                )
                nc.scalar.copy(
                    out=yTb[b][:, tq * 128 : (tq + 1) * 128], in_=py[:, 128:256]
                )

            def mlp_block(b):
                cols = slice(b * BW, (b + 1) * BW)
                h_sb = []
                for j in range(2):
                    ph = ps_m.tile([128, BW], f32, tag="pm")
                    for kc in range(3):
                        rhs = (
                            yTa[b][:, :]
                            if kc == 0
                            else yTb[b][:, :]
                            if kc == 1
                            else xskT_sb[:, cols]
                        )
                        nc.tensor.matmul(
                            out=ph[:, :],
                            lhsT=w1_sb[:, kc, j * 128 : (j + 1) * 128],
                            rhs=rhs,
                            start=(kc == 0),
                            stop=(kc == 2),
                        )
                    hj = mpool.tile([128, BW], f32, tag="h")
                    nc.scalar.activation(
                        out=hj[:, :], in_=ph[:, :], func=Act.Relu,
                        bias=b1_sb[:, j : j + 1],
                    )
                    h_sb.append(hj)
                for j2 in range(2):
                    po = ps_m.tile([128, BW], f32, tag="pm")
                    for kh in range(2):
                        nc.tensor.matmul(
                            out=po[:, :],
                            lhsT=w2_sb[:, kh, j2 * 128 : (j2 + 1) * 128],
                            rhs=h_sb[kh][:, :],
                            start=(kh == 0),
                            stop=(kh == 1),
                        )
                    ot = mpool.tile([128, BW], f32, tag="ot")
                    nc.scalar.activation(
                        out=ot[:, :], in_=po[:, :], func=Act.Identity,
                        bias=b2_sb[:, j2 : j2 + 1],
                    )
                    nc.sync.dma_start(
                        out=outT_d[j2 * 128 : (j2 + 1) * 128, cols], in_=ot[:, :]
                    )

            for t in range(QT):
                scan_tile(t)
                if (t + 1) * 128 % BW == 0:
                    mlp_block((t + 1) * 128 // BW - 1)

    nc.compile()
    return nc


def _host_prep(x, pos, x_skip, pos_skip, W1, b1, W2, b2):
    """Build the per-core input maps (all float32, layouts device expects)."""
    x = np.ascontiguousarray(np.asarray(x, dtype=np.float32))
    pos = np.asarray(pos, dtype=np.float32)
    x_skip = np.asarray(x_skip, dtype=np.float32)
    pos_skip = np.asarray(pos_skip, dtype=np.float32)
    W1 = np.ascontiguousarray(np.asarray(W1, dtype=np.float32))
    W2 = np.ascontiguousarray(np.asarray(W2, dtype=np.float32))
    b1 = np.asarray(b1, dtype=np.float32)
    b2 = np.asarray(b2, dtype=np.float32)

    import ml_dtypes

    bf = ml_dtypes.bfloat16

    def split3(v):
        h = v.astype(bf).astype(np.float32)
        r = v - h
        l1 = r.astype(bf).astype(np.float32)
        l2 = (r - l1).astype(bf).astype(np.float32)
        return h, l1, l2

    # moving table [128, N]: rows are p-side components for the 6 exact
    # cross-term groups (hh, h-l1, l1-h, h-l2, l2-h, l1-l1) plus split pp
    pp = np.sum(pos.astype(np.float64) * pos.astype(np.float64), axis=1).astype(
        np.float32
    )
    ph, pl1, pl2 = split3(pos)  # [N, 3] each
    pph, ppl1, ppl2 = split3(pp)  # [N]
    p4 = np.zeros((128, N), dtype=bf)
    p_rows = [ph, pl1, ph, pl2, ph, pl1]
    for g in range(6):
        p4[3 * g : 3 * g + 3, :] = p_rows[g].T.astype(bf)
    p4[18, :] = pph.astype(bf)
    p4[19, :] = ppl1.astype(bf)
    p4[20, :] = ppl2.astype(bf)
    p4 = np.ascontiguousarray(p4)

    b1_dev = np.ascontiguousarray(b1.reshape(2, 128).T)  # [128, 2]
    b2_dev = np.ascontiguousarray(b2.reshape(2, 128).T)

    ident_host = np.ascontiguousarray(np.eye(128, dtype=np.float32))
    eps2_host = np.full((128, 1), EPS2, dtype=np.float32)
    in_maps = []
    for c in range(NCORES):
        r0, r1 = c * MS, (c + 1) * MS
        ps = pos_skip[r0:r1]
        qh, ql1, ql2 = split3(2.0 * ps)  # [MS, 3] each
        q4 = np.zeros((128, MS), dtype=bf)
        q_rows = [qh, qh, ql1, qh, ql2, ql1]
        for g in range(6):
            q4[3 * g : 3 * g + 3, :] = q_rows[g].T.astype(bf)
        q4[18:21, :] = np.float32(-1.0)
        q4 = np.ascontiguousarray(q4)
        qq = np.ascontiguousarray(
            (np.sum(ps * ps, axis=1) - EPS2).reshape(QT, 128).T
        )  # [128, QT], minus the reciprocal-guard epsilon
        xskT = np.ascontiguousarray(x_skip[r0:r1].T)  # [C_SKIP, MS]
        in_maps.append(
            {
                "x": x,
                "p4": p4,
                "q4": q4,
                "qq": qq,
                "xskT": xskT,
                "W1": W1,
                "b1": b1_dev,
                "W2": W2,
                "b2": b2_dev,
                "ident": ident_host,
                "eps2": eps2_host,
            }
        )
    return in_maps


def run(inputs, trace=False):
    """Run the bass kernel on 8 cores. Returns (out [M, C_IN... H], results)."""
    from concourse.bass_utils import run_bass_kernel_spmd

    if "nc" not in _prog_cache:
        _prog_cache["nc"] = _build_program()
    nc = _prog_cache["nc"]

    in_maps = _host_prep(
        inputs["x"], inputs["pos"], inputs["x_skip"], inputs["pos_skip"],
        inputs["W1"], inputs["b1"], inputs["W2"], inputs["b2"],
    )
    try:
        res = run_bass_kernel_spmd(
            nc, in_maps, core_ids=list(range(NCORES)), trace=trace
        )
    except Exception:
        # transient axon device wedge (NRT_EXEC_UNIT_UNRECOVERABLE) --
        # reset the terminal-side NRT state and retry once
        try:
            import ctypes

            lib = ctypes.CDLL("/opt/axon/libaxon_pjrt.so")
            lib.axon_reset.restype = ctypes.c_int64
            lib.axon_reset()
        except Exception:
            pass
        res = run_bass_kernel_spmd(
            nc, in_maps, core_ids=list(range(NCORES)), trace=False
        )
    out = np.concatenate(
        [np.ascontiguousarray(r["outT"].T) for r in res.results], axis=0
    )
    return out, res


def kernel(x, pos, batch, x_skip, pos_skip, batch_skip, W1, b1, W2, b2):
    out, _ = run(
        {
            "x": x, "pos": pos, "x_skip": x_skip, "pos_skip": pos_skip,
            "W1": W1, "b1": b1, "W2": W2, "b2": b2,
        }
    )
    return (
        out,
        np.asarray(pos_skip, dtype=np.float32),
        np.asarray(batch_skip),
    )
